# revision 23
# baseline (speedup 1.0000x reference)
"""Multi-head causal attention with RoPE on 8 Trainium2 NeuronCores.

Reference computation (B=2, T=2048, C=1024, H=16, Dh=64, fp32):
    qkv = x @ w_qkv + b_qkv ; split q,k,v ; RoPE(q), RoPE(k)
    attn = softmax_causal(q k^T / sqrt(Dh)) @ v ; out = attn @ w_proj + b_proj

Sharding: core c = b*4 + g handles batch b and head group g (heads 4g..4g+3).
Data-parallel over batch, tensor-parallel over heads (w_qkv column-split,
w_proj row-split).  Each core emits a partial [T, C] projection output; the
host sums the 4 per-batch partials and adds b_proj.

Per-core kernel, one merged pipeline (no phase barriers):
  - QK projection for head pair (0,1) first (kc-accumulated per span tile),
    bias folded into the ACT-Identity PSUM->SBUF evacuation as a
    per-partition bias vector, RoPE on DVE in bf16 (2x mode) with the
    partition rotate done by a 128x128 permutation matmul.
  - attention for heads (0,1) starts immediately; V tiles and the QK
    projection + RoPE for heads (2,3) are interleaved into the PE stream as
    filler so the PE never idles while ACT runs the exp stream.
  - S matmuls for the two heads of a pair run CONCURRENTLY in the PE array
    (row-packed: K=64 each, tile_position rows 0-63 / 64-127), writing the
    two halves of one [128, 1024] 2-bank PSUM tile; ONE ACT exp covers both.
  - causal: per k-tile the q-range is narrowed; the diagonal 128-block gets
    -400 added via a triangular-mask matmul before exp.
  - PV per head with a ones-augmented V (65th column) so the matmul also
    emits the softmax denominator; normalize = DVE reciprocal_approx_fast on
    the denominator row + DMA partition-broadcast + one DVE multiply
    (no ACT, no table switches -- ACT stays 100% on the exp stream).
  - attention for heads (2,3) runs with the output projection of each
    finished span interleaved as PE filler; outputs DMA per token tile.
  - all ACT functions (Exp/Identity) are pinned to one activation-table set
    so the table is loaded exactly once.
"""

import types
from collections import deque

import numpy as np
import ml_dtypes

import concourse.bacc as bacc
import concourse.bass as bass
import concourse.mybir as mybir
from concourse.tile import TileContext
from concourse.bass_utils import run_bass_kernel_spmd
from concourse.hw_specs import get_activation_tables
import bass_rust as _bass_rust

F32 = mybir.dt.float32
BF16 = mybir.dt.bfloat16
NPBF16 = np.dtype(ml_dtypes.bfloat16)

B, T, C = 2, 2048, 1024
H, DH = 16, 64
GH = 4  # heads per core
N_CORES = 8
NCHUNK = C // 128  # 8 contraction chunks
NT = T // 128  # 16 token tiles
NSPAN = T // 512  # 4 query spans
QK_COLS = 2 * GH * DH  # 512 = q cols (256) + k cols (256)
VA = GH * (DH + 1)  # 260 = v cols augmented with ones column per head
EXP = mybir.ActivationFunctionType.Exp
IDENT = mybir.ActivationFunctionType.Identity

_ONE_SET = "natural_log_exp_and_others"
_PINNED = {
    mybir.ActivationFunctionType.Exp,
    mybir.ActivationFunctionType.Ln,
    mybir.ActivationFunctionType.Identity,
    mybir.ActivationFunctionType.Copy,
}


def _patched_insert_act_table_loads(self):
    """Pin Exp/Ln/Identity/Copy to the one table set containing all of them,
    so the kernel pays exactly one ACT_TABLE_LOAD (the default chooser picks
    a different set per function and reloads on every switch)."""
    has_activation = any(
        isinstance(i, mybir.InstActivation)
        for b in self.main_func.blocks
        for i in b.instructions
    )
    if not has_activation:
        return
    tables = []
    for name, fns in get_activation_tables(self.m.arch).items():
        if name != _ONE_SET:
            fns = fns - _PINNED
        tables.append((name, fns))
    _bass_rust.insert_act_table_loads(self, tables)


def _build():
    nc = bacc.Bacc("TRN2", target_bir_lowering=False, debug=False, num_devices=N_CORES)
    nc.insert_act_table_loads = types.MethodType(_patched_insert_act_table_loads, nc)

    xT = nc.dram_tensor("xT", [C, T], BF16, kind="ExternalInput")
    wqk = nc.dram_tensor("wqk", [C, QK_COLS], BF16, kind="ExternalInput")
    wv = nc.dram_tensor("wv", [C, VA], BF16, kind="ExternalInput")
    bqkT_d = nc.dram_tensor("bqkT", [128, 4], F32, kind="ExternalInput")
    bv_d = nc.dram_tensor("bv", [1, VA], BF16, kind="ExternalInput")
    cos_d = nc.dram_tensor("cosT", [128, T], BF16, kind="ExternalInput")
    sinp_d = nc.dram_tensor("sinTp", [128, T], BF16, kind="ExternalInput")
    perm_d = nc.dram_tensor("perm", [128, 128], BF16, kind="ExternalInput")
    maskT_d = nc.dram_tensor("maskT", [128, 128], BF16, kind="ExternalInput")
    id_d = nc.dram_tensor("id128", [128, 128], BF16, kind="ExternalInput")
    wproj_d = nc.dram_tensor("wproj", [2, 128, C], BF16, kind="ExternalInput")
    out_d = nc.dram_tensor("out", [T, C], BF16, kind="ExternalOutput")

    with TileContext(nc) as tc:
        with tc.tile_pool(name="persist", bufs=1) as pers:
            ones = pers.tile([1, 512], BF16, tag="ones")
            nc.vector.memset(ones, 1.0)

            # ---- input DMAs, spread across engine queues --------------
            xt = []
            for kc in range(NCHUNK):
                t = pers.tile([128, T], BF16, tag="xt", bufs=NCHUNK, name=f"xt{kc}")
                xt.append(t)
            wqk_t = []
            for kc in range(NCHUNK):
                t = pers.tile(
                    [128, QK_COLS], BF16, tag="wqk", bufs=NCHUNK, name=f"wqk{kc}"
                )
                wqk_t.append(t)
            wv_t = []
            for kc in range(NCHUNK):
                t = pers.tile([128, VA], BF16, tag="wv", bufs=NCHUNK, name=f"wv{kc}")
                wv_t.append(t)
            cos_sb = pers.tile([128, T], BF16, tag="cos")
            sinp_sb = pers.tile([128, T], BF16, tag="sinp")
            perm_sb = pers.tile([128, 128], BF16, tag="perm")
            mask_sb = pers.tile([128, 128], BF16, tag="maskT")
            id_sb = pers.tile([128, 128], BF16, tag="id128")
            bqkT_sb = pers.tile([128, 4], F32, tag="bqkT")
            bv_sb = pers.tile([1, VA], BF16, tag="bv")
            wproj_sb = []
            for p in range(2):
                t = pers.tile([128, C], BF16, tag="wproj", bufs=2, name=f"wproj{p}")
                wproj_sb.append(t)

            # interleave wqk ahead of each x chunk on the same queue so
            # QK matmul kc is gated only by its own chunk's arrival;
            # chunks 6,7 ride the gpsimd queue behind the rope tables.
            for kc in range(6):
                eng = nc.sync if kc % 2 == 0 else nc.scalar
                eng.dma_start(out=wqk_t[kc], in_=wqk[128 * kc : 128 * (kc + 1), :])
                eng.dma_start(out=xt[kc], in_=xT[128 * kc : 128 * (kc + 1), :])
            nc.gpsimd.dma_start(out=cos_sb, in_=cos_d[:, :])
            nc.gpsimd.dma_start(out=sinp_sb, in_=sinp_d[:, :])
            nc.gpsimd.dma_start(out=perm_sb, in_=perm_d[:, :])
            nc.gpsimd.dma_start(out=mask_sb, in_=maskT_d[:, :])
            nc.gpsimd.dma_start(out=id_sb, in_=id_d[:, :])
            nc.gpsimd.dma_start(out=bqkT_sb, in_=bqkT_d[:, :])
            nc.gpsimd.dma_start(out=bv_sb, in_=bv_d[:, :])
            for kc in range(6, NCHUNK):
                nc.gpsimd.dma_start(
                    out=wqk_t[kc], in_=wqk[128 * kc : 128 * (kc + 1), :]
                )
                nc.gpsimd.dma_start(out=xt[kc], in_=xT[128 * kc : 128 * (kc + 1), :])
            for kc in range(NCHUNK):
                nc.gpsimd.dma_start(out=wv_t[kc], in_=wv[128 * kc : 128 * (kc + 1), :])
            for p in range(2):
                nc.gpsimd.dma_start(out=wproj_sb[p], in_=wproj_d[p, :, :])

            # ---- persistent compute tiles -----------------------------
            qkt = []  # Q heads(0,1), Q(2,3), K(0,1), K(2,3) as [128, T]
            for i in range(4):
                t = pers.tile([128, T], BF16, tag="qkt", bufs=4, name=f"qkt{i}")
                qkt.append(t)
            vaug = []  # 16 tiles [128, VA], k-tile-major natural layout V
            for j in range(NT):
                t = pers.tile([128, VA], BF16, tag="vaug", bufs=NT, name=f"vaug{j}")
                vaug.append(t)
            attn = []  # normalized attn^T for head pairs
            for p in range(2):
                t = pers.tile([128, T], BF16, tag="attn", bufs=2, name=f"attn{p}")
                attn.append(t)
            # rope + normalize sbuf scratch
            qkb_pool = pers  # reuse persist pool with rotating tags

            # ---------- emitters --------------------------------------
            rope_pending = deque()

            def qk_stage1(ct, sp, ps_pool):
                """8 accumulation matmuls + biased ACT evacuation + sin-mul."""
                pq = ps_pool.tile([128, 512], F32, tag="qk", name="psqk")
                ss = slice(512 * sp, 512 * (sp + 1))
                for kc in range(NCHUNK):
                    nc.tensor.matmul(
                        pq,
                        wqk_t[kc][:, 128 * ct : 128 * (ct + 1)],
                        xt[kc][:, ss],
                        start=(kc == 0),
                        stop=(kc == NCHUNK - 1),
                    )
                qkb = qkb_pool.tile([128, 512], BF16, tag="qkb", bufs=4, name="qkb")
                nc.scalar.activation(
                    out=qkb, in_=pq, func=IDENT, bias=bqkT_sb[:, ct : ct + 1]
                )
                t2 = qkb_pool.tile([128, 512], BF16, tag="t2", bufs=4, name="t2")
                nc.vector.tensor_mul(t2, qkb, sinp_sb[:, ss])
                rope_pending.append((ct, ss, qkb, t2))

            def qk_stage2(aux_pool):
                """permute matmul + cos-mul + add (one lag behind stage1)."""
                ct, ss, qkb, t2 = rope_pending.popleft()
                pp = aux_pool.tile([128, 512], F32, tag="aux", name="psperm")
                nc.tensor.matmul(pp, perm_sb, t2, start=True, stop=True)
                nc.vector.tensor_mul(qkt[ct][:, ss], qkb, cos_sb[:, ss])
                nc.vector.tensor_add(qkt[ct][:, ss], qkt[ct][:, ss], pp)

            def v_tile(j, aux_pool):
                pvx = aux_pool.tile([128, 512], F32, tag="aux", name="psv")
                reg = pvx[:, 0:VA]
                ts = slice(128 * j, 128 * (j + 1))
                for kc in range(NCHUNK):
                    nc.tensor.matmul(
                        reg, xt[kc][:, ts], wv_t[kc], start=(kc == 0), stop=False
                    )
                nc.tensor.matmul(reg, ones[0:1, 0:128], bv_sb, start=False, stop=True)
                nc.vector.tensor_copy(vaug[j], reg)

            def att_pair(p, s_pool, pv_pool, et_pool, filler, span_done, counter):
                """Attention for head pair p (heads 2p, 2p+1), row-packed S,
                shared exp, per-head PV with ones-augmented V.

                `filler` holds (ready_at_chunk, emit_fn) pairs consumed one
                per chunk once ready -- PE work to fill ACT-paced gaps
                without ever heading the PE queue with an unready op."""
                qt, kt = qkt[p], qkt[2 + p]
                pend = deque()
                norm_pend = deque()
                pvt = {}

                def norm(half, s, pv):
                    # 1/denom = exp(-ln(denom)): Ln/Exp share one ACT table
                    # set (pinned), then broadcast across partitions on the
                    # idle GPSIMD engine.
                    rln = qkb_pool.tile([1, 512], F32, tag="rln", bufs=4, name="rln")
                    nc.scalar.activation(
                        out=rln, in_=pv[64:65, :], func=mybir.ActivationFunctionType.Ln
                    )
                    r = qkb_pool.tile([1, 512], F32, tag="rcp", bufs=4, name="rcp")
                    nc.scalar.activation(out=r, in_=rln, func=EXP, scale=-1.0)
                    rb = qkb_pool.tile([64, 512], F32, tag="rb", bufs=4, name="rb")
                    nc.gpsimd.partition_broadcast(rb, r[0:1, :], channels=64)
                    po = half * 64
                    nc.vector.tensor_mul(
                        attn[p][po : po + 64, 512 * s : 512 * (s + 1)],
                        pv[0:64, :],
                        rb,
                    )

                def emit_pv(item):
                    # defer the normalize by a chunk: its Ln would otherwise
                    # head the in-order ACT queue while the final PV matmul
                    # is still working through the PE queue, stalling the
                    # exp stream.
                    j, s, q0, w, et = item
                    last = j == 4 * s + 3
                    for half in (0, 1):
                        lh = 2 * p + half
                        nc.tensor.matmul(
                            pvt[(half, s)][0:65, q0 - 512 * s : q0 - 512 * s + w],
                            vaug[j][:, 65 * lh : 65 * (lh + 1)],
                            et[:, 512 * half : 512 * half + w],
                            start=(j == 0),
                            stop=last,
                        )
                        if last:
                            norm_pend.append((half, s, pvt.pop((half, s))))

                for s in range(NSPAN):
                    for half in (0, 1):
                        pvt[(half, s)] = pv_pool.tile(
                            [65, 512], F32, tag="pv", name=f"pspv{half}_{s}"
                        )
                    for j in range(4 * s + 4):
                        q0 = max(512 * s, 128 * j)
                        w = 512 * (s + 1) - q0
                        diag = j // 4 == s
                        sp_t = s_pool.tile([128, 1024], F32, tag="spair", name="pss")
                        for half in (0, 1):
                            po = half * 64
                            nc.tensor.matmul(
                                sp_t[:, 512 * half : 512 * half + w],
                                kt[po : po + 64, 128 * j : 128 * (j + 1)],
                                qt[po : po + 64, q0 : q0 + w],
                                start=True,
                                stop=not diag,
                            )
                            if diag:
                                nc.tensor.matmul(
                                    sp_t[:, 512 * half : 512 * half + 128],
                                    mask_sb,
                                    id_sb,
                                    start=False,
                                    stop=True,
                                )
                        et = et_pool.tile(
                            [128, 1024], BF16, tag="et", bufs=6, name="et"
                        )
                        nc.scalar.activation(out=et, in_=sp_t, func=EXP, scale=0.125)
                        pend.append((j, s, q0, w, et))
                        for _ in range(len(norm_pend)):
                            half, ns_, pv = norm_pend.popleft()
                            norm(half, ns_, pv)
                            if half == 1:
                                span_done(ns_)
                        if len(pend) > 2:
                            emit_pv(pend.popleft())
                        counter["i"] += 1
                        if filler and filler[0][0] <= counter["i"]:
                            filler.popleft()[1]()
                while pend:
                    emit_pv(pend.popleft())
                while norm_pend:
                    half, ns_, pv = norm_pend.popleft()
                    norm(half, ns_, pv)
                    if half == 1:
                        span_done(ns_)
                while filler:
                    filler.popleft()[1]()

            # ---------- scope 1: QK for heads (0,1) + V tiles 0-3 ------
            with tc.tile_pool(name="s1ps", bufs=4, space="PSUM") as s1:
                with tc.tile_pool(name="s1aux", bufs=2, space="PSUM") as s1aux:
                    for sp in range(NSPAN):
                        for ct in (2, 0):  # K cols first, then Q
                            qk_stage1(ct, sp, s1)
                            if len(rope_pending) > 1:
                                qk_stage2(s1aux)
                    while rope_pending:
                        qk_stage2(s1aux)
                    for j in range(4):
                        v_tile(j, s1aux)

            # ---------- scope 2: attention ----------------------------
            with (
                tc.tile_pool(name="att", bufs=2, space="PSUM") as att_ps,
                tc.tile_pool(name="aux2", bufs=1, space="PSUM") as aux2,
            ):
                # pair (0,1) with V4-15 + QK23 as PE filler
                with tc.tile_pool(name="qk23", bufs=1, space="PSUM") as qk23ps:
                    filler = deque()
                    for j in range(4, 8):
                        filler.append((0, lambda j=j: v_tile(j, aux2)))
                    for sp in range(NSPAN):
                        for ct in (3, 1):
                            filler.append(
                                (0, lambda ct=ct, sp=sp: qk_stage1(ct, sp, qk23ps))
                            )
                            filler.append((0, lambda: qk_stage2(aux2)))
                        if sp == 1:
                            for j in range(8, 12):
                                filler.append((0, lambda j=j: v_tile(j, aux2)))
                    for j in range(12, NT):
                        filler.append((0, lambda j=j: v_tile(j, aux2)))
                    c01 = {"i": 0}
                    att_pair(0, att_ps, att_ps, pers, filler, lambda s: None, c01)

                # pair (2,3) with per-span projection + output as filler
                with tc.tile_pool(name="proj", bufs=1, space="PSUM") as proj_ps:
                    filler23 = deque()

                    def proj_half(it, nh):
                        # alternate between the proj bank and the (now idle)
                        # aux bank for an effective double buffer
                        if (2 * it + nh) % 2 == 0:
                            pj = proj_ps.tile([128, 512], F32, tag="proj", name="psproj")
                        else:
                            pj = aux2.tile([128, 512], F32, tag="aux", name="psproj")
                        ts = slice(128 * it, 128 * (it + 1))
                        ns = slice(512 * nh, 512 * (nh + 1))
                        for p in range(2):
                            nc.tensor.matmul(
                                pj,
                                attn[p][:, ts],
                                wproj_sb[p][:, ns],
                                start=(p == 0),
                                stop=(p == 1),
                            )
                        ob = qkb_pool.tile(
                            [128, 512], BF16, tag="ob", bufs=4, name="ob"
                        )
                        nc.vector.tensor_copy(ob, pj)
                        nc.sync.dma_start(out=out_d[ts, ns], in_=ob)

                    c23 = {"i": 0}

                    def span_done(s):
                        # release 4 chunks later so the first proj matmul
                        # never heads the PE queue before the normalize
                        # chain (ln -> exp -> broadcast -> mul) lands
                        base = c23["i"] + 6
                        k = 0
                        for it in range(4 * s, 4 * s + 4):
                            for nh in range(2):
                                filler23.append(
                                    (base + k, lambda it=it, nh=nh: proj_half(it, nh))
                                )
                                k += 1

                    att_pair(1, att_ps, att_ps, pers, filler23, span_done, c23)
                    while filler23:
                        filler23.popleft()[1]()

    nc.compile()
    return nc


_NC = None


def _get_nc():
    global _NC
    if _NC is None:
        _NC = _build()
    return _NC


def _rope_tables():
    theta = (10000.0 ** (-np.arange(0, DH, 2, dtype=np.float32) / DH)).astype(
        np.float32
    )
    t = np.arange(T, dtype=np.float32)
    sinusoid = np.outer(t, theta).astype(np.float32)  # [T, DH/2]
    sin = np.concatenate([np.sin(sinusoid), np.sin(sinusoid)], axis=1)  # [T, DH]
    cos = np.concatenate([np.cos(sinusoid), np.cos(sinusoid)], axis=1)
    cosT = cos.T  # [DH, T]
    sinT = sin.T
    # sin_perm[e] = sin[(e+32) % 64]
    idx = (np.arange(DH) + 32) % DH
    sinTp = sinT[idx]
    cos2 = np.ascontiguousarray(np.concatenate([cosT, cosT], axis=0))  # [128, T]
    sinp2 = np.ascontiguousarray(np.concatenate([sinTp, sinTp], axis=0))
    return cos2, sinp2


def _perm_matrix():
    p = np.zeros((128, 128), dtype=np.float32)
    for m in range(128):
        blk = m // 64
        k = blk * 64 + (m % 64 + 32) % 64
        p[k, m] = 1.0
    return p


def _mask_matrices():
    # maskT.T @ I adds -400 to S^T[k, q] where k > q (then exp(0.125*s)=0):
    # maskT[a, b] = -400 where b > a
    maskT = -400.0 * np.triu(np.ones((128, 128), dtype=np.float32), 1)
    return maskT, np.eye(128, dtype=np.float32)


def _bf(a):
    return np.ascontiguousarray(np.asarray(a, dtype=np.float32).astype(NPBF16))


def _prepare_in_maps(x, w_qkv, b_qkv, w_proj):
    x = np.asarray(x, dtype=np.float32)
    w_qkv = np.asarray(w_qkv, dtype=np.float32)
    b_qkv = np.asarray(b_qkv, dtype=np.float32)
    w_proj = np.asarray(w_proj, dtype=np.float32)

    cos2, sinp2 = _rope_tables()
    perm = _bf(_perm_matrix())
    maskT, id128 = _mask_matrices()
    maskT, id128 = _bf(maskT), _bf(id128)
    xTs = [_bf(x[b].T) for b in range(B)]
    cos2, sinp2 = _bf(cos2), _bf(sinp2)

    in_maps = []
    for c in range(N_CORES):
        b, g = divmod(c, 4)
        h0 = g * GH  # first head of the group
        qcols = w_qkv[:, h0 * DH : (h0 + GH) * DH]
        kcols = w_qkv[:, C + h0 * DH : C + (h0 + GH) * DH]
        wqk = _bf(np.concatenate([qcols, kcols], axis=1))
        wv = np.zeros((C, VA), dtype=np.float32)
        bv = np.zeros((1, VA), dtype=np.float32)
        for j in range(GH):
            src = 2 * C + (h0 + j) * DH
            wv[:, j * 65 : j * 65 + DH] = w_qkv[:, src : src + DH]
            bv[0, j * 65 : j * 65 + DH] = b_qkv[src : src + DH]
            bv[0, j * 65 + DH] = 1.0
        bqk = np.concatenate(
            [b_qkv[h0 * DH : (h0 + GH) * DH], b_qkv[C + h0 * DH : C + (h0 + GH) * DH]]
        ).astype(np.float32)
        bqkT = np.ascontiguousarray(bqk.reshape(4, 128).T)  # [128, ct]
        wproj = np.stack(
            [w_proj[(h0 + 2 * p) * DH : (h0 + 2 * p + 2) * DH, :] for p in range(2)]
        )
        in_maps.append(
            {
                "xT": xTs[b],
                "wqk": wqk,
                "wv": _bf(wv),
                "bqkT": bqkT,
                "bv": _bf(bv),
                "cosT": cos2,
                "sinTp": sinp2,
                "perm": perm,
                "maskT": maskT,
                "id128": id128,
                "wproj": _bf(wproj),
            }
        )
    return in_maps


def run(x, w_qkv, b_qkv, w_proj, b_proj, trace=False, tmpdir=None):
    nc = _get_nc()
    in_maps = _prepare_in_maps(x, w_qkv, b_qkv, w_proj)
    res = run_bass_kernel_spmd(
        nc, in_maps, list(range(N_CORES)), trace=trace, tmpdir=tmpdir
    )
    b_proj = np.asarray(b_proj, dtype=np.float32)
    out = np.empty((B, T, C), dtype=np.float32)
    for b in range(B):
        acc = res.results[4 * b]["out"].astype(np.float32)
        for g in range(1, 4):
            acc = acc + res.results[4 * b + g]["out"].astype(np.float32)
        out[b] = acc + b_proj
    return out, res


def kernel(x, w_qkv, b_qkv, w_proj, b_proj):
    out, _ = run(x, w_qkv, b_qkv, w_proj, b_proj, trace=False)
    return out


# revision 24
# speedup vs baseline: 1.1267x; 1.1267x over previous
"""Multi-head causal attention with RoPE on 8 Trainium2 NeuronCores.

Reference computation (B=2, T=2048, C=1024, H=16, Dh=64, fp32):
    qkv = x @ w_qkv + b_qkv ; split q,k,v ; RoPE(q), RoPE(k)
    attn = softmax_causal(q k^T / sqrt(Dh)) @ v ; out = attn @ w_proj + b_proj

Sharding: core c = b*4 + g handles batch b and head group g (heads 4g..4g+3).
Data-parallel over batch, tensor-parallel over heads (w_qkv column-split,
w_proj row-split).  Each core emits a partial [T, C] projection output; the
host sums the 4 per-batch partials and adds b_proj.

Per-core kernel, one merged pipeline (no phase barriers):
  - QK projection for head pair (0,1) first (kc-accumulated per span tile),
    bias folded into the ACT-Identity PSUM->SBUF evacuation as a
    per-partition bias vector, RoPE on DVE in bf16 (2x mode) with the
    partition rotate done by a 128x128 permutation matmul.
  - attention for heads (0,1) starts immediately; V tiles and the QK
    projection + RoPE for heads (2,3) are interleaved into the PE stream as
    filler so the PE never idles while ACT runs the exp stream.
  - S matmuls for the two heads of a pair run CONCURRENTLY in the PE array
    (row-packed: K=64 each, tile_position rows 0-63 / 64-127), writing the
    two halves of one [128, 1024] 2-bank PSUM tile; ONE ACT exp covers both.
  - causal: per k-tile the q-range is narrowed; the diagonal 128-block gets
    -400 added via a triangular-mask matmul before exp.
  - PV per head with a ones-augmented V (65th column) so the matmul also
    emits the softmax denominator; normalize = DVE reciprocal_approx_fast on
    the denominator row + DMA partition-broadcast + one DVE multiply
    (no ACT, no table switches -- ACT stays 100% on the exp stream).
  - attention for heads (2,3) runs with the output projection of each
    finished span interleaved as PE filler; outputs DMA per token tile.
  - all ACT functions (Exp/Identity) are pinned to one activation-table set
    so the table is loaded exactly once.
"""

import types
from collections import deque

import numpy as np
import ml_dtypes

import concourse.bacc as bacc
import concourse.bass as bass
import concourse.mybir as mybir
from concourse.tile import TileContext
from concourse.bass_utils import run_bass_kernel_spmd
from concourse.hw_specs import get_activation_tables
import bass_rust as _bass_rust

F32 = mybir.dt.float32
BF16 = mybir.dt.bfloat16
NPBF16 = np.dtype(ml_dtypes.bfloat16)

B, T, C = 2, 2048, 1024
H, DH = 16, 64
GH = 4  # heads per core
N_CORES = 8
NCHUNK = C // 128  # 8 contraction chunks
NT = T // 128  # 16 token tiles
NSPAN = T // 512  # 4 query spans
QK_COLS = 2 * GH * DH  # 512 = q cols (256) + k cols (256)
VA = GH * (DH + 1)  # 260 = v cols augmented with ones column per head
EXP = mybir.ActivationFunctionType.Exp
IDENT = mybir.ActivationFunctionType.Identity

_ONE_SET = "natural_log_exp_and_others"
_PINNED = {
    mybir.ActivationFunctionType.Exp,
    mybir.ActivationFunctionType.Ln,
    mybir.ActivationFunctionType.Identity,
    mybir.ActivationFunctionType.Copy,
}


def _patched_insert_act_table_loads(self):
    """Pin Exp/Ln/Identity/Copy to the one table set containing all of them,
    so the kernel pays exactly one ACT_TABLE_LOAD (the default chooser picks
    a different set per function and reloads on every switch)."""
    has_activation = any(
        isinstance(i, mybir.InstActivation)
        for b in self.main_func.blocks
        for i in b.instructions
    )
    if not has_activation:
        return
    tables = []
    for name, fns in get_activation_tables(self.m.arch).items():
        if name != _ONE_SET:
            fns = fns - _PINNED
        tables.append((name, fns))
    _bass_rust.insert_act_table_loads(self, tables)


def _build():
    nc = bacc.Bacc("TRN2", target_bir_lowering=False, debug=False, num_devices=N_CORES)
    nc.insert_act_table_loads = types.MethodType(_patched_insert_act_table_loads, nc)

    xT = nc.dram_tensor("xT", [C, T], BF16, kind="ExternalInput")
    wqk = nc.dram_tensor("wqk", [C, QK_COLS], BF16, kind="ExternalInput")
    wv = nc.dram_tensor("wv", [C, VA], BF16, kind="ExternalInput")
    bqkT_d = nc.dram_tensor("bqkT", [128, 4], F32, kind="ExternalInput")
    bv_d = nc.dram_tensor("bv", [1, VA], BF16, kind="ExternalInput")
    cos_d = nc.dram_tensor("cosT", [128, T], BF16, kind="ExternalInput")
    sinp_d = nc.dram_tensor("sinTp", [128, T], BF16, kind="ExternalInput")
    perm_d = nc.dram_tensor("perm", [128, 128], BF16, kind="ExternalInput")
    maskT_d = nc.dram_tensor("maskT", [128, 128], BF16, kind="ExternalInput")
    id_d = nc.dram_tensor("id128", [128, 128], BF16, kind="ExternalInput")
    wproj_d = nc.dram_tensor("wproj", [2, 128, C], BF16, kind="ExternalInput")
    out_d = nc.dram_tensor("out", [T, C], BF16, kind="ExternalOutput")

    with TileContext(nc) as tc:
        with tc.tile_pool(name="persist", bufs=1) as pers:
            ones = pers.tile([1, 512], BF16, tag="ones")
            nc.vector.memset(ones, 1.0)

            # ---- input DMAs, spread across engine queues --------------
            xt = []
            for kc in range(NCHUNK):
                t = pers.tile([128, T], BF16, tag="xt", bufs=NCHUNK, name=f"xt{kc}")
                xt.append(t)
            wqk_t = []
            for kc in range(NCHUNK):
                t = pers.tile(
                    [128, QK_COLS], BF16, tag="wqk", bufs=NCHUNK, name=f"wqk{kc}"
                )
                wqk_t.append(t)
            wv_t = []
            for kc in range(NCHUNK):
                t = pers.tile([128, VA], BF16, tag="wv", bufs=NCHUNK, name=f"wv{kc}")
                wv_t.append(t)
            cos_sb = pers.tile([128, T], BF16, tag="cos")
            sinp_sb = pers.tile([128, T], BF16, tag="sinp")
            perm_sb = pers.tile([128, 128], BF16, tag="perm")
            mask_sb = pers.tile([128, 128], BF16, tag="maskT")
            id_sb = pers.tile([128, 128], BF16, tag="id128")
            bqkT_sb = pers.tile([128, 4], F32, tag="bqkT")
            bv_sb = pers.tile([1, VA], BF16, tag="bv")
            wproj_sb = []
            for p in range(2):
                t = pers.tile([128, C], BF16, tag="wproj", bufs=2, name=f"wproj{p}")
                wproj_sb.append(t)

            # interleave wqk ahead of each x chunk on the same queue so
            # QK matmul kc is gated only by its own chunk's arrival;
            # chunks 6,7 ride the gpsimd queue behind the rope tables.
            for kc in range(6):
                eng = nc.sync if kc % 2 == 0 else nc.scalar
                eng.dma_start(out=wqk_t[kc], in_=wqk[128 * kc : 128 * (kc + 1), :])
                eng.dma_start(out=xt[kc], in_=xT[128 * kc : 128 * (kc + 1), :])
            nc.gpsimd.dma_start(out=cos_sb, in_=cos_d[:, :])
            nc.gpsimd.dma_start(out=sinp_sb, in_=sinp_d[:, :])
            nc.gpsimd.dma_start(out=perm_sb, in_=perm_d[:, :])
            nc.gpsimd.dma_start(out=mask_sb, in_=maskT_d[:, :])
            nc.gpsimd.dma_start(out=id_sb, in_=id_d[:, :])
            nc.gpsimd.dma_start(out=bqkT_sb, in_=bqkT_d[:, :])
            nc.gpsimd.dma_start(out=bv_sb, in_=bv_d[:, :])
            for kc in range(6, NCHUNK):
                nc.gpsimd.dma_start(
                    out=wqk_t[kc], in_=wqk[128 * kc : 128 * (kc + 1), :]
                )
                nc.gpsimd.dma_start(out=xt[kc], in_=xT[128 * kc : 128 * (kc + 1), :])
            for kc in range(NCHUNK):
                nc.gpsimd.dma_start(out=wv_t[kc], in_=wv[128 * kc : 128 * (kc + 1), :])
            for p in range(2):
                nc.gpsimd.dma_start(out=wproj_sb[p], in_=wproj_d[p, :, :])

            # ---- persistent compute tiles -----------------------------
            qkt = []  # Q heads(0,1), Q(2,3), K(0,1), K(2,3) as [128, T]
            for i in range(4):
                t = pers.tile([128, T], BF16, tag="qkt", bufs=4, name=f"qkt{i}")
                qkt.append(t)
            vaug = []  # 16 tiles [128, VA], k-tile-major natural layout V
            for j in range(NT):
                t = pers.tile([128, VA], BF16, tag="vaug", bufs=NT, name=f"vaug{j}")
                vaug.append(t)
            attn = []  # normalized attn^T for head pairs
            for p in range(2):
                t = pers.tile([128, T], BF16, tag="attn", bufs=2, name=f"attn{p}")
                attn.append(t)
            # rope + normalize sbuf scratch
            qkb_pool = pers  # reuse persist pool with rotating tags

            # ---------- emitters --------------------------------------
            rope_pending = deque()

            def qk_tail(ct, sp, pq):
                """biased ACT evacuation + sin-mul for one QK span tile."""
                ss = slice(512 * sp, 512 * (sp + 1))
                qkb = qkb_pool.tile([128, 512], BF16, tag="qkb", bufs=4, name="qkb")
                nc.scalar.activation(
                    out=qkb, in_=pq, func=IDENT, bias=bqkT_sb[:, ct : ct + 1]
                )
                t2 = qkb_pool.tile([128, 512], BF16, tag="t2", bufs=4, name="t2")
                nc.vector.tensor_mul(t2, qkb, sinp_sb[:, ss])
                rope_pending.append((ct, ss, qkb, t2))

            def qk_stage1(ct, sp, ps_pool):
                """8 accumulation matmuls + evacuation (filler form)."""
                pq = ps_pool.tile([128, 512], F32, tag="qk", name="psqk")
                ss = slice(512 * sp, 512 * (sp + 1))
                for kc in range(NCHUNK):
                    nc.tensor.matmul(
                        pq,
                        wqk_t[kc][:, 128 * ct : 128 * (ct + 1)],
                        xt[kc][:, ss],
                        start=(kc == 0),
                        stop=(kc == NCHUNK - 1),
                    )
                qk_tail(ct, sp, pq)

            def qk_stage2(aux_pool):
                """permute matmul + cos-mul + add (one lag behind stage1)."""
                ct, ss, qkb, t2 = rope_pending.popleft()
                pp = aux_pool.tile([128, 512], F32, tag="aux", name="psperm")
                nc.tensor.matmul(pp, perm_sb, t2, start=True, stop=True)
                nc.vector.tensor_mul(qkt[ct][:, ss], qkb, cos_sb[:, ss])
                nc.vector.tensor_add(qkt[ct][:, ss], qkt[ct][:, ss], pp)

            def v_tile(j, aux_pool):
                pvx = aux_pool.tile([128, 512], F32, tag="aux", name="psv")
                reg = pvx[:, 0:VA]
                ts = slice(128 * j, 128 * (j + 1))
                for kc in range(NCHUNK):
                    nc.tensor.matmul(
                        reg, xt[kc][:, ts], wv_t[kc], start=(kc == 0), stop=False
                    )
                nc.tensor.matmul(reg, ones[0:1, 0:128], bv_sb, start=False, stop=True)
                nc.vector.tensor_copy(vaug[j], reg)

            def att_pair(p, s_pool, pv_pool, et_pool, filler, span_done, counter):
                """Attention for head pair p (heads 2p, 2p+1), row-packed S,
                shared exp, per-head PV with ones-augmented V.

                `filler` holds (ready_at_chunk, emit_fn) pairs consumed one
                per chunk once ready -- PE work to fill ACT-paced gaps
                without ever heading the PE queue with an unready op."""
                qt, kt = qkt[p], qkt[2 + p]
                pend = deque()
                norm_pend = deque()
                pvt = {}

                def norm(half, s, pv):
                    # 1/denom = exp(-ln(denom)): Ln/Exp share one ACT table
                    # set (pinned), then broadcast across partitions on the
                    # idle GPSIMD engine.
                    rln = qkb_pool.tile([1, 512], F32, tag="rln", bufs=4, name="rln")
                    nc.scalar.activation(
                        out=rln, in_=pv[64:65, :], func=mybir.ActivationFunctionType.Ln
                    )
                    r = qkb_pool.tile([1, 512], F32, tag="rcp", bufs=4, name="rcp")
                    nc.scalar.activation(out=r, in_=rln, func=EXP, scale=-1.0)
                    rb = qkb_pool.tile([64, 512], F32, tag="rb", bufs=4, name="rb")
                    nc.gpsimd.partition_broadcast(rb, r[0:1, :], channels=64)
                    po = half * 64
                    nc.vector.tensor_mul(
                        attn[p][po : po + 64, 512 * s : 512 * (s + 1)],
                        pv[0:64, :],
                        rb,
                    )

                def emit_pv(item):
                    # defer the normalize by a chunk: its Ln would otherwise
                    # head the in-order ACT queue while the final PV matmul
                    # is still working through the PE queue, stalling the
                    # exp stream.
                    j, s, q0, w, et = item
                    last = j == 4 * s + 3
                    for half in (0, 1):
                        lh = 2 * p + half
                        nc.tensor.matmul(
                            pvt[(half, s)][0:65, q0 - 512 * s : q0 - 512 * s + w],
                            vaug[j][:, 65 * lh : 65 * (lh + 1)],
                            et[:, 512 * half : 512 * half + w],
                            start=(j == 0),
                            stop=last,
                        )
                        if last:
                            norm_pend.append((half, s, pvt.pop((half, s))))

                for s in range(NSPAN):
                    for half in (0, 1):
                        pvt[(half, s)] = pv_pool.tile(
                            [65, 512], F32, tag="pv", name=f"pspv{half}_{s}"
                        )
                    for j in range(4 * s + 4):
                        q0 = max(512 * s, 128 * j)
                        w = 512 * (s + 1) - q0
                        diag = j // 4 == s
                        sp_t = s_pool.tile([128, 1024], F32, tag="spair", name="pss")
                        for half in (0, 1):
                            po = half * 64
                            nc.tensor.matmul(
                                sp_t[:, 512 * half : 512 * half + w],
                                kt[po : po + 64, 128 * j : 128 * (j + 1)],
                                qt[po : po + 64, q0 : q0 + w],
                                start=True,
                                stop=not diag,
                            )
                            if diag:
                                nc.tensor.matmul(
                                    sp_t[:, 512 * half : 512 * half + 128],
                                    mask_sb,
                                    id_sb,
                                    start=False,
                                    stop=True,
                                )
                        et = et_pool.tile(
                            [128, 1024], BF16, tag="et", bufs=6, name="et"
                        )
                        nc.scalar.activation(out=et, in_=sp_t, func=EXP, scale=0.125)
                        pend.append((j, s, q0, w, et))
                        for _ in range(len(norm_pend)):
                            half, ns_, pv = norm_pend.popleft()
                            norm(half, ns_, pv)
                            if half == 1:
                                span_done(ns_)
                        if len(pend) > 2:
                            emit_pv(pend.popleft())
                        counter["i"] += 1
                        if filler and filler[0][0] <= counter["i"]:
                            filler.popleft()[1]()
                while pend:
                    emit_pv(pend.popleft())
                while norm_pend:
                    half, ns_, pv = norm_pend.popleft()
                    norm(half, ns_, pv)
                    if half == 1:
                        span_done(ns_)
                while filler:
                    filler.popleft()[1]()

            # ---------- scope 1: QK for heads (0,1) + V tiles 0-3 ------
            # kc-outer over all 8 (ct, sp) accumulators: the PE consumes
            # each x chunk the moment its DMA lands, and all QK(0,1) tiles
            # complete together right after the last chunk arrives.
            with tc.tile_pool(name="s1qk", bufs=8, space="PSUM") as s1qk:
                pq8 = {}
                for ct in (2, 0):
                    for sp in range(NSPAN):
                        pq8[(ct, sp)] = s1qk.tile(
                            [128, 512], F32, tag="qk", name=f"psqk{ct}_{sp}"
                        )
                for kc in range(NCHUNK):
                    for ct in (2, 0):
                        for sp in range(NSPAN):
                            nc.tensor.matmul(
                                pq8[(ct, sp)],
                                wqk_t[kc][:, 128 * ct : 128 * (ct + 1)],
                                xt[kc][:, 512 * sp : 512 * (sp + 1)],
                                start=(kc == 0),
                                stop=(kc == NCHUNK - 1),
                            )
                for ct in (2, 0):  # K tiles first so attention unblocks
                    for sp in range(NSPAN):
                        qk_tail(ct, sp, pq8[(ct, sp)])
            with tc.tile_pool(name="s1aux", bufs=2, space="PSUM") as s1aux:
                while rope_pending:
                    qk_stage2(s1aux)
                for j in range(4):
                    v_tile(j, s1aux)

            # ---------- scope 2: attention ----------------------------
            with (
                tc.tile_pool(name="att", bufs=2, space="PSUM") as att_ps,
                tc.tile_pool(name="aux2", bufs=1, space="PSUM") as aux2,
            ):
                # pair (0,1) with V4-15 + QK23 as PE filler
                with tc.tile_pool(name="qk23", bufs=1, space="PSUM") as qk23ps:
                    filler = deque()
                    for j in range(4, 8):
                        filler.append((0, lambda j=j: v_tile(j, aux2)))
                    for sp in range(NSPAN):
                        for ct in (3, 1):
                            filler.append(
                                (0, lambda ct=ct, sp=sp: qk_stage1(ct, sp, qk23ps))
                            )
                            filler.append((0, lambda: qk_stage2(aux2)))
                        if sp == 1:
                            for j in range(8, 12):
                                filler.append((0, lambda j=j: v_tile(j, aux2)))
                    for j in range(12, NT):
                        filler.append((0, lambda j=j: v_tile(j, aux2)))
                    c01 = {"i": 0}
                    att_pair(0, att_ps, att_ps, pers, filler, lambda s: None, c01)

                # pair (2,3) with per-span projection + output as filler
                with tc.tile_pool(name="proj", bufs=1, space="PSUM") as proj_ps:
                    filler23 = deque()

                    def proj_half(it, nh):
                        # alternate between the proj bank and the (now idle)
                        # aux bank for an effective double buffer
                        if (2 * it + nh) % 2 == 0:
                            pj = proj_ps.tile([128, 512], F32, tag="proj", name="psproj")
                        else:
                            pj = aux2.tile([128, 512], F32, tag="aux", name="psproj")
                        ts = slice(128 * it, 128 * (it + 1))
                        ns = slice(512 * nh, 512 * (nh + 1))
                        for p in range(2):
                            nc.tensor.matmul(
                                pj,
                                attn[p][:, ts],
                                wproj_sb[p][:, ns],
                                start=(p == 0),
                                stop=(p == 1),
                            )
                        ob = qkb_pool.tile(
                            [128, 512], BF16, tag="ob", bufs=4, name="ob"
                        )
                        nc.vector.tensor_copy(ob, pj)
                        nc.sync.dma_start(out=out_d[ts, ns], in_=ob)

                    c23 = {"i": 0}

                    def span_done(s):
                        # release 4 chunks later so the first proj matmul
                        # never heads the PE queue before the normalize
                        # chain (ln -> exp -> broadcast -> mul) lands
                        base = c23["i"] + 6
                        k = 0
                        for it in range(4 * s, 4 * s + 4):
                            for nh in range(2):
                                filler23.append(
                                    (base + k, lambda it=it, nh=nh: proj_half(it, nh))
                                )
                                k += 1

                    att_pair(1, att_ps, att_ps, pers, filler23, span_done, c23)
                    while filler23:
                        filler23.popleft()[1]()

    nc.compile()
    return nc


_NC = None


def _get_nc():
    global _NC
    if _NC is None:
        _NC = _build()
    return _NC


def _rope_tables():
    theta = (10000.0 ** (-np.arange(0, DH, 2, dtype=np.float32) / DH)).astype(
        np.float32
    )
    t = np.arange(T, dtype=np.float32)
    sinusoid = np.outer(t, theta).astype(np.float32)  # [T, DH/2]
    sin = np.concatenate([np.sin(sinusoid), np.sin(sinusoid)], axis=1)  # [T, DH]
    cos = np.concatenate([np.cos(sinusoid), np.cos(sinusoid)], axis=1)
    cosT = cos.T  # [DH, T]
    sinT = sin.T
    # sin_perm[e] = sin[(e+32) % 64]
    idx = (np.arange(DH) + 32) % DH
    sinTp = sinT[idx]
    cos2 = np.ascontiguousarray(np.concatenate([cosT, cosT], axis=0))  # [128, T]
    sinp2 = np.ascontiguousarray(np.concatenate([sinTp, sinTp], axis=0))
    return cos2, sinp2


def _perm_matrix():
    p = np.zeros((128, 128), dtype=np.float32)
    for m in range(128):
        blk = m // 64
        k = blk * 64 + (m % 64 + 32) % 64
        p[k, m] = 1.0
    return p


def _mask_matrices():
    # maskT.T @ I adds -400 to S^T[k, q] where k > q (then exp(0.125*s)=0):
    # maskT[a, b] = -400 where b > a
    maskT = -400.0 * np.triu(np.ones((128, 128), dtype=np.float32), 1)
    return maskT, np.eye(128, dtype=np.float32)


def _bf(a):
    return np.ascontiguousarray(np.asarray(a, dtype=np.float32).astype(NPBF16))


def _prepare_in_maps(x, w_qkv, b_qkv, w_proj):
    x = np.asarray(x, dtype=np.float32)
    w_qkv = np.asarray(w_qkv, dtype=np.float32)
    b_qkv = np.asarray(b_qkv, dtype=np.float32)
    w_proj = np.asarray(w_proj, dtype=np.float32)

    cos2, sinp2 = _rope_tables()
    perm = _bf(_perm_matrix())
    maskT, id128 = _mask_matrices()
    maskT, id128 = _bf(maskT), _bf(id128)
    xTs = [_bf(x[b].T) for b in range(B)]
    cos2, sinp2 = _bf(cos2), _bf(sinp2)

    in_maps = []
    for c in range(N_CORES):
        b, g = divmod(c, 4)
        h0 = g * GH  # first head of the group
        qcols = w_qkv[:, h0 * DH : (h0 + GH) * DH]
        kcols = w_qkv[:, C + h0 * DH : C + (h0 + GH) * DH]
        wqk = _bf(np.concatenate([qcols, kcols], axis=1))
        wv = np.zeros((C, VA), dtype=np.float32)
        bv = np.zeros((1, VA), dtype=np.float32)
        for j in range(GH):
            src = 2 * C + (h0 + j) * DH
            wv[:, j * 65 : j * 65 + DH] = w_qkv[:, src : src + DH]
            bv[0, j * 65 : j * 65 + DH] = b_qkv[src : src + DH]
            bv[0, j * 65 + DH] = 1.0
        bqk = np.concatenate(
            [b_qkv[h0 * DH : (h0 + GH) * DH], b_qkv[C + h0 * DH : C + (h0 + GH) * DH]]
        ).astype(np.float32)
        bqkT = np.ascontiguousarray(bqk.reshape(4, 128).T)  # [128, ct]
        wproj = np.stack(
            [w_proj[(h0 + 2 * p) * DH : (h0 + 2 * p + 2) * DH, :] for p in range(2)]
        )
        in_maps.append(
            {
                "xT": xTs[b],
                "wqk": wqk,
                "wv": _bf(wv),
                "bqkT": bqkT,
                "bv": _bf(bv),
                "cosT": cos2,
                "sinTp": sinp2,
                "perm": perm,
                "maskT": maskT,
                "id128": id128,
                "wproj": _bf(wproj),
            }
        )
    return in_maps


def run(x, w_qkv, b_qkv, w_proj, b_proj, trace=False, tmpdir=None):
    nc = _get_nc()
    in_maps = _prepare_in_maps(x, w_qkv, b_qkv, w_proj)
    res = run_bass_kernel_spmd(
        nc, in_maps, list(range(N_CORES)), trace=trace, tmpdir=tmpdir
    )
    b_proj = np.asarray(b_proj, dtype=np.float32)
    out = np.empty((B, T, C), dtype=np.float32)
    for b in range(B):
        acc = res.results[4 * b]["out"].astype(np.float32)
        for g in range(1, 4):
            acc = acc + res.results[4 * b + g]["out"].astype(np.float32)
        out[b] = acc + b_proj
    return out, res


def kernel(x, w_qkv, b_qkv, w_proj, b_proj):
    out, _ = run(x, w_qkv, b_qkv, w_proj, b_proj, trace=False)
    return out
